# revision 12
# baseline (speedup 1.0000x reference)
"""Trainium2 Bass kernel for nn_CMIConnector: visual->ds projection, linear SSM
scan along Lv with time-invariant per-(batch,channel) gates, then out-projection
to d_model. Data-parallel over batch across 8 NeuronCores (2 rows/core).

Reference math (per batch row b):
    tc     = mean_Lt(text_embeds[b])                    # [Dt]
    delta  = sigmoid(tc @ Wd.T + bd)                    # [ds]
    B_vec  = tc @ WB.T + bB                             # [ds]
    C_vec  = tc @ WC.T + bC                             # [ds]
    x_t    = visual[b, t] @ Wx.T + bx                   # [ds]
    h_t    = (1-delta) * h_{t-1} + delta*B_vec*x_t      # linear scan over Lv
    out_t  = (C_vec * h_t) @ Wo.T + bo                  # [dm]

HBM-bandwidth bound: per core the fp16 output (64 MiB) dominates against
~358 GB/s of HBM. Precision ladder, all inside the 2e-2 rel-err gate
(measured end-to-end 1.34e-2): output fp16 (host upcasts), weights/text
fp16, and the visual input fp8 e3m4 -- its ~1.3e-2 quantization error is
the budget spend that halves the biggest load (16.8 -> 8.4 MiB/core) and
halves visual SBUF, funding a third output staging buffer.

Dataflow per core, against the ~250us DMA floor:
  * All loads are issued eagerly as a handful of large DMAs (two packed
    weight/bias images, text, 4 fp8 visual windows) and complete in the
    first ~35us at full rate, before the store stream ramps.
  * Output stores are grouped GRP=8 d_model chunks -> one 4 MiB DMA per
    group (4 per half-Lv window) on the Sync HWDGE ring; 3 staging buffers
    so a group's evacuation never waits on a 2-groups-ago store completing.
  * Fine-grained interleave: one x-proj+scan chunk of the NEXT window is
    emitted after each of this window's store-groups (last chunk one group
    early), so the store queue always has a group in flight while the PE
    does A-phase work and y(w+1) is ready with no boundary stall.

The out-projection runs TRANSPOSED (Wo.T stationary, scan output y as the
fp16 moving operand); output tiles come out [d_model-chunk, time] into a
grouped DRAM layout the host un-permutes. Everything ds-sized is DUPLICATED
onto both 64-partition halves: gate and x-proj matmuls issue column-tiled
twins ((0,0)/(0,64), concurrent in the PE array), the scan runs on all 128
partitions, and the out-projection row-packs TWO d_model chunks as
(0,0)/(64,0) tile pairs that execute concurrently -- halving PE time per
output tile so the PE stays off the critical path even at the cold
(1.2 GHz) HAM clock.

PSUM evacuation (ScalarE/VectorE are the only PSUM readers) adds bias bo
and casts to fp16 in one op. Each psA/psB pair splits across BOTH engines
so they drain concurrently (a block-modulo split alternates them serially
and paces the PE at single-engine rate -- measured 2x slower); the tile
pattern runs 9:7 toward the faster ScalarE to balance VectorE's extra scan
load, with the doubled-engine slot at a group's FIRST set so the group's
final tile never waits on a serialized pair. One unified PSUM pool
(4 x [128,1024] = all 8 banks) serves x-proj and out-proj.
"""

import os
import sys

import ml_dtypes
import numpy as np

for _p in ("/opt/trn_rl_repo",):
    if _p not in sys.path and os.path.isdir(_p):
        sys.path.insert(0, _p)

import concourse.bass as bass  # noqa: E402
import concourse.tile as tile  # noqa: E402
from concourse import bacc, mybir  # noqa: E402
from concourse.bass_utils import run_bass_kernel_spmd  # noqa: E402

F32 = mybir.dt.float32
F8E3 = mybir.dt.float8e3
F32R = mybir.dt.float32r
FP16 = mybir.dt.float16

# Problem shapes (hardcoded per the contract).
B, Lv, Dv = 16, 4096, 1024
Lt, Dt = 128, 4096
DS, DM = 64, 4096
NCORES = 8
BPC = B // NCORES  # batches per core

MM_DTYPE = F32R  # kept for test-harness compat; the big matmuls run fp16

NJ = Dt // 128  # gate contraction chunks
ND = Dv // 128  # x-proj contraction chunks
NH = 2  # halves of Lv (pipeline window = one half)
HLEN = Lv // NH
NTH = HLEN // 512  # 512-wide time chunks per half
NMC = DM // 128  # out-proj d_model chunks
GRP = 8  # d_model chunks coalesced per output store (4 MiB per DMA)
NG = NMC // GRP  # store groups per window (== NTH for the 1:1 interleave)


def _build_program(mm_dtype=MM_DTYPE):
    nc = bacc.Bacc()
    AF = mybir.ActivationFunctionType
    OP = mybir.AluOpType

    # All weight/text tensors are host-packed into their on-chip layouts so
    # every load is one large DMA with contiguous per-partition rows.
    vis16 = nc.dram_tensor("vis16", [BPC, NH, 128, ND * HLEN], F8E3, kind="ExternalInput")
    # both batches' text concatenated along the free dim: [128, NJ, BPC*Lt]
    text16 = nc.dram_tensor("text16", [128, NJ, BPC * Lt], FP16, kind="ExternalInput")
    # packed weights: cols 0:ND*DS = Wx.T chunks, cols ND*DS: = Wo.T row-dup
    wxo16 = nc.dram_tensor("wxo16", [128, ND * DS + DM], FP16, kind="ExternalInput")
    wg16 = nc.dram_tensor("wg16", [128, NJ, 3, DS], FP16, kind="ExternalInput")
    # packed f32 per-partition columns: bd, -bd, bB, bC, bx, then bo (NMC cols)
    biasf = nc.dram_tensor("biasf", [128, 5 + NMC], F32, kind="ExternalInput")
    # grouped output: element (b, h, g, p, j*HLEN+t) =
    #   out_fp16(batch b, dm=(g*GRP+j)*128+p, time=h*HLEN+t)
    outG16 = nc.dram_tensor(
        "outG16", [BPC, NH, NG, 128, GRP * HLEN], FP16, kind="ExternalOutput"
    )

    with tile.TileContext(nc) as tc:
        with (
            tc.tile_pool(name="persist", bufs=1) as persist,
            tc.tile_pool(name="visb", bufs=3) as visb,
        ):
            vts = {}

            def load_vis(b, h):
                vt = visb.tile([128, ND * HLEN], F8E3, tag="v", name="vt")
                nc.scalar.dma_start(out=vt[:], in_=vis16[b, h])
                vts[(b, h)] = vt

            # ---- eager preloads: two packed DMAs replace eight tiny ones
            # so the DMA ramp is short and the ring never idles at t=0 ----
            wxo_sb = persist.tile([128, ND * DS + DM], FP16)
            nc.scalar.dma_start(out=wxo_sb[:], in_=wxo16[:])
            bf_sb = persist.tile([128, 5 + NMC], F32)
            nc.scalar.dma_start(out=bf_sb[:], in_=biasf[:])
            bd_sb = bf_sb[:, 0:1]
            nbd_sb = bf_sb[:, 1:2]
            bb_sb = bf_sb[:, 2:3]
            bc_sb = bf_sb[:, 3:4]
            bx_sb = bf_sb[:, 4:5]

            delta_sb = persist.tile([128, BPC], F32)
            a_sb = persist.tile([128, BPC], F32)
            bv_sb = persist.tile([128, BPC], F32)
            cv_sb = persist.tile([128, BPC], F32)
            db_sb = persist.tile([128, BPC], F32)
            cdb_sb = persist.tile([128, BPC], F32)
            cdbx_sb = persist.tile([128, BPC], F32)

            # ---- Phase 0: fused text-mean gate projections (fp16 PE) ----
            # Both batches' tokens ride in one moving operand (N=BPC*Lt); every
            # z is computed twice via column-tiled twin matmuls so the whole
            # gate chain lives on all 128 partitions (rows 64-127 copy 0-63),
            # which the scan and out-proj packing rely on. Temporaries live in
            # a scoped pool that frees before the big main-loop pools open.
            with (
                tc.tile_pool(name="gatep", bufs=1) as gatep,
                tc.tile_pool(name="psum0", bufs=1, space="PSUM") as psum0,
            ):
                wg_sb = gatep.tile([128, NJ, 3, DS], FP16)
                nc.scalar.dma_start(out=wg_sb[:], in_=wg16[:])
                ttc = gatep.tile([128, NJ, BPC * Lt], FP16)
                nc.scalar.dma_start(out=ttc[:], in_=text16[:])
                # visual for the first two windows + out-proj weights: queued
                # behind the gate inputs, in first-use order.
                load_vis(0, 0)
                load_vis(0, 1)

                zd_sb = gatep.tile([128, BPC], F32)
                zb_sb = gatep.tile([128, BPC], F32)
                zc_sb = gatep.tile([128, BPC], F32)
                zd_ps = psum0.tile([128, BPC * Lt], F32, tag="zd")
                zb_ps = psum0.tile([128, BPC * Lt], F32, tag="zb")
                zc_ps = psum0.tile([128, BPC * Lt], F32, tag="zc")
                for j in range(NJ):
                    for g, ps in enumerate((zd_ps, zb_ps, zc_ps)):
                        for lo in (0, DS):
                            nc.tensor.matmul(
                                ps[lo : lo + DS, :],
                                wg_sb[:, j, g, :],
                                ttc[:, j, :],
                                start=(j == 0),
                                stop=(j == NJ - 1),
                            )
                for b in range(BPC):
                    bsl = slice(b * Lt, (b + 1) * Lt)
                    # mean over Lt (1/Lt folded into wg16 on host)
                    nc.vector.reduce_sum(
                        zd_sb[:, b : b + 1], zd_ps[:, bsl], axis=mybir.AxisListType.X
                    )
                    nc.vector.reduce_sum(
                        zb_sb[:, b : b + 1], zb_ps[:, bsl], axis=mybir.AxisListType.X
                    )
                    nc.vector.reduce_sum(
                        zc_sb[:, b : b + 1], zc_ps[:, bsl], axis=mybir.AxisListType.X
                    )

                nc.scalar.activation(
                    delta_sb[:], zd_sb[:], AF.Sigmoid, bias=bd_sb[:, 0:1], scale=1.0
                )
                nc.scalar.activation(
                    a_sb[:], zd_sb[:], AF.Sigmoid, bias=nbd_sb[:, 0:1], scale=-1.0
                )
                nc.vector.tensor_scalar_add(bv_sb[:], zb_sb[:], bb_sb[:, 0:1])
                nc.vector.tensor_scalar_add(cv_sb[:], zc_sb[:], bc_sb[:, 0:1])
                nc.vector.tensor_mul(db_sb[:], delta_sb[:], bv_sb[:])
                # Fold the output gate C into the scan input: scanning
                # u'_t = C*delta*B*x_t yields y_t = C*h_t directly.
                nc.vector.tensor_mul(cdb_sb[:], db_sb[:], cv_sb[:])
                nc.vector.tensor_scalar_mul(cdbx_sb[:], cdb_sb[:], bx_sb[:, 0:1])

            # ---- Phases 1+2: x-proj + chunked scan (A), out-proj (B) ----
            with (
                tc.tile_pool(name="psall", bufs=4, space="PSUM") as psall,
                tc.tile_pool(name="ubp", bufs=4) as ubp,
                tc.tile_pool(name="ybp", bufs=2) as ybp,
                tc.tile_pool(name="abp", bufs=2) as abp,
                tc.tile_pool(name="outp", bufs=3) as outp,
            ):
                ys, abcs = {}, {}

                def phase_a_begin(b, h):
                    if h == 0:
                        # broadcast decay gate a=(1-delta); the scan consumes
                        # the same [128, 512] columns every chunk.
                        a_bc = abp.tile([128, 512], F32, tag="a", name="a_bc")
                        nc.gpsimd.memset(a_bc[:], 1.0)
                        nc.vector.tensor_scalar_mul(
                            a_bc[:], a_bc[:], a_sb[:, b : b + 1]
                        )
                        abcs[b] = a_bc
                    ys[(b, h)] = ybp.tile([128, HLEN], FP16, tag="y", name="y_r")

                def phase_a_chunk(b, h, i):
                    y_r, a_bc, vt = ys[(b, h)], abcs[b], vts[(b, h)]
                    sl = slice(i * 512, (i + 1) * 512)
                    xp = psall.tile([128, 1024], F32, tag="op", name="xp")
                    for d in range(ND):
                        for lo in (0, DS):  # col-tiled twins, concurrent
                            nc.tensor.matmul(
                                xp[lo : lo + DS, 0:512],
                                wxo_sb[:, d * DS : (d + 1) * DS],
                                vt[:, d * HLEN + i * 512 : d * HLEN + (i + 1) * 512],
                                start=(d == 0),
                                stop=(d == ND - 1),
                            )
                    # u = (C*deltaB) * x_raw + (C*deltaB)*bx
                    u_c = ubp.tile([128, 512], F32, tag="u", name="u_c")
                    nc.scalar.activation(
                        u_c[:],
                        xp[:, 0:512],
                        AF.Identity,
                        bias=cdbx_sb[:, b : b + 1],
                        scale=cdb_sb[:, b : b + 1],
                    )
                    # chunked scan; chain via the previous chunk's last col
                    if i == 0:
                        init = 0.0 if h == 0 else ys[(b, 0)][:, HLEN - 1 : HLEN]
                    else:
                        init = y_r[:, i * 512 - 1 : i * 512]
                    nc.vector.tensor_tensor_scan(
                        y_r[:, sl],
                        a_bc[:],
                        u_c[:],
                        init,
                        OP.mult,
                        OP.add,
                    )

                def phase_a(b, h):
                    phase_a_begin(b, h)
                    for i in range(NTH):
                        phase_a_chunk(b, h, i)

                # PSUM evacuation (f32->fp16 + bias bo), fine-grained tile
                # interleave across ScalarE/VectorE so both engines drain PSUM
                # concurrently. 9:7 toward the faster ScalarE (0.93 vs 1.07
                # ns/elem) balances against VectorE's extra scan load; the
                # double-S lands at a group's FIRST set so the group-final
                # tile never waits on a serialized pair.
                EVAC_PAT = [1,1,0,1,0,1,0,1,0,1,0,1,0,1,0,1]  # 1=ScalarE
                evac_i = [0]

                def evac(ps, dst, mc):
                    if EVAC_PAT[evac_i[0] % 16]:
                        nc.scalar.activation(
                            dst, ps[:], AF.Identity,
                            bias=bf_sb[:, 5 + mc : 6 + mc], scale=1.0,
                        )
                    else:
                        nc.vector.tensor_scalar_add(
                            dst, ps[:], bf_sb[:, 5 + mc : 6 + mc]
                        )
                    evac_i[0] += 1

                def phase_b_group(b, h, g):
                    y_r = ys[(b, h)]
                    og = outp.tile([128, GRP * HLEN], FP16, tag="og", name="og")
                    for p in range(GRP // 2):
                        mcA = g * GRP + 2 * p
                        mcB = g * GRP + 2 * p + 1
                        for q in range(HLEN // 1024):
                            psA = psall.tile([128, 1024], F32, tag="op", name="psA")
                            psB = psall.tile([128, 1024], F32, tag="op", name="psB")
                            for hh in range(2):
                                sl = slice(q * 1024 + hh * 512, q * 1024 + (hh + 1) * 512)
                                osl = slice(hh * 512, (hh + 1) * 512)
                                # row-packed pair: (0,0) and (64,0) tiles run
                                # concurrently; each LDW overlaps the other
                                # row-group's running matmul.
                                nc.tensor.matmul(
                                    psA[:, osl],
                                    wxo_sb[0:DS, ND * DS + mcA * 128 : ND * DS + (mcA + 1) * 128],
                                    y_r[0:DS, sl],
                                    start=True,
                                    stop=True,
                                )
                                nc.tensor.matmul(
                                    psB[:, osl],
                                    wxo_sb[DS:128, ND * DS + mcB * 128 : ND * DS + (mcB + 1) * 128],
                                    y_r[DS:128, sl],
                                    start=True,
                                    stop=True,
                                )
                            base = 2 * p * HLEN + q * 1024
                            evac(psA, og[:, base : base + 1024], mcA)
                            evac(psB, og[:, base + HLEN : base + HLEN + 1024], mcB)
                    nc.sync.dma_start(out=outG16[b, h, g], in_=og[:])

                # Fine-grained interleave: one x-proj/scan chunk of the NEXT
                # window rides after every other one of this window's 2 MiB
                # store-groups, so the store queue always has groups in flight
                # while the PE does phase-A work. The last chunk is emitted
                # two groups early so y(w+1) completes while groups NG-2/NG-1
                # are still storing.
                windows = [(0, 0), (0, 1), (1, 0), (1, 1)]
                load_vis(1, 0)
                phase_a(0, 0)
                for w, (b, h) in enumerate(windows):
                    nxt = windows[w + 1] if w + 1 < len(windows) else None
                    if nxt:
                        phase_a_begin(*nxt)
                    for g in range(NG):
                        phase_b_group(b, h, g)
                        if nxt:
                            if g == 0 and w > 0 and w + 2 < len(windows):
                                load_vis(*windows[w + 2])
                            if g < NG - 2:
                                phase_a_chunk(*nxt, g)
                            elif g == NG - 2:
                                phase_a_chunk(*nxt, NG - 2)
                                phase_a_chunk(*nxt, NG - 1)
    return nc


def _prep_host_inputs(
    visual_feats, text_embeds, Wx, bx, Wd, bd, WB, bB, WC, bC, Wo, bo
):
    f = lambda a: np.asarray(a, dtype=np.float32)
    # [B, Lv, Dv] -> [B, NH, 128p, ND*HLEN] fp16 with element
    # (b, h, p, d*HLEN+t) = visual[b, h*HLEN+t, d*128+p]
    vis16 = np.ascontiguousarray(
        f(visual_feats)
        .transpose(0, 2, 1)
        .reshape(B, ND, 128, NH, HLEN)
        .transpose(0, 3, 2, 1, 4)
        .reshape(B, NH, 128, ND * HLEN)
        .astype(ml_dtypes.float8_e3m4)
    )
    # [B, Lt, Dt] -> per-core [128p, NJ, BPC*Lt] with Dt index = j*128 + p
    text16 = np.ascontiguousarray(
        f(text_embeds)
        .transpose(0, 2, 1)
        .reshape(B, NJ, 128, Lt)
        .transpose(0, 2, 1, 3)
        .astype(np.float16)
    )
    # Wx.T [Dv, ds] -> [128p, ND*ds] with Dv index = c*128 + p, then Wo.T
    # row-dup appended: one packed fp16 weight image, one DMA on device.
    wxt16 = (
        f(Wx).T.reshape(ND, 128, DS).transpose(1, 0, 2).reshape(128, ND * DS)
        .astype(np.float16)
    )
    # Gate weights transposed, pre-scaled by 1/Lt (the text mean), fp16,
    # packed [Dt, 3, ds] -> [128p, NJ, 3, ds] with Dt index = j*128 + p.
    wg16 = np.ascontiguousarray(
        (np.stack([f(Wd).T, f(WB).T, f(WC).T], axis=1) / np.float32(Lt))
        .reshape(NJ, 128, 3, DS)
        .transpose(1, 0, 2, 3)
        .astype(np.float16)
    )
    wot = f(Wo).T.astype(np.float16)  # [ds, dm]
    wott16 = np.concatenate([wot, wot], axis=0)  # [128, dm]
    wxo16 = np.ascontiguousarray(np.concatenate([wxt16, wott16], axis=1))
    # bo -> [128p, NMC] with dm index = mc*128 + p
    bo_t = f(bo).reshape(NMC, 128).T
    dup = lambda a: np.concatenate([f(a).reshape(-1, 1)] * 2, axis=0)
    biasf = np.ascontiguousarray(
        np.concatenate(
            [dup(bd), dup(-f(bd)), dup(bB), dup(bC), dup(bx), bo_t], axis=1
        ).astype(np.float32)
    )
    shared = {
        "wxo16": wxo16,
        "wg16": wg16,
        "biasf": biasf,
    }
    in_maps = []
    for c in range(NCORES):
        m = dict(shared)
        m["vis16"] = np.ascontiguousarray(vis16[c * BPC : (c + 1) * BPC])
        # concat the core's batches along the token axis: [128, NJ, BPC*Lt]
        m["text16"] = np.ascontiguousarray(
            np.concatenate(
                [text16[c * BPC + b] for b in range(BPC)], axis=2
            )
        )
        in_maps.append(m)
    return in_maps


_PROGRAM_CACHE = {}


def _get_program(mm_dtype=MM_DTYPE):
    key = str(mm_dtype)
    if key not in _PROGRAM_CACHE:
        nc = _build_program(mm_dtype)
        if not nc.is_finalized():
            nc.finalize()
        _PROGRAM_CACHE[key] = nc
    return _PROGRAM_CACHE[key]


def run(inputs: dict, trace: bool = False, mm_dtype=MM_DTYPE):
    """Run the kernel on all 8 cores; returns (full_output, BassKernelResults)."""
    nc = _get_program(mm_dtype)
    in_maps = _prep_host_inputs(**inputs)
    res = run_bass_kernel_spmd(nc, in_maps, list(range(NCORES)), trace=trace)
    # outG16 is [BPC, NH, NG, 128, GRP*HLEN] fp16 per core; un-permute + upcast
    # on host: (b, h, g, p, j, t) -> (b, (h,t), (g,j,p)).
    full = np.concatenate(
        [
            res.results[i]["outG16"]
            .reshape(BPC, NH, NG, 128, GRP, HLEN)
            .transpose(0, 1, 5, 2, 4, 3)
            .reshape(BPC, Lv, DM)
            .astype(np.float32)
            for i in range(NCORES)
        ],
        axis=0,
    )
    return np.ascontiguousarray(full), res


def kernel(**inputs) -> np.ndarray:
    out, _ = run(inputs, trace=False)
    return out


# revision 13
# speedup vs baseline: 1.0232x; 1.0232x over previous
"""Trainium2 Bass kernel for nn_CMIConnector: visual->ds projection, linear SSM
scan along Lv with time-invariant per-(batch,channel) gates, then out-projection
to d_model. Data-parallel over batch across 8 NeuronCores (2 rows/core).

Reference math (per batch row b):
    tc     = mean_Lt(text_embeds[b])                    # [Dt]
    delta  = sigmoid(tc @ Wd.T + bd)                    # [ds]
    B_vec  = tc @ WB.T + bB                             # [ds]
    C_vec  = tc @ WC.T + bC                             # [ds]
    x_t    = visual[b, t] @ Wx.T + bx                   # [ds]
    h_t    = (1-delta) * h_{t-1} + delta*B_vec*x_t      # linear scan over Lv
    out_t  = (C_vec * h_t) @ Wo.T + bo                  # [dm]

HBM-bandwidth bound: per core the fp16 output (64 MiB) dominates against
~358 GB/s of HBM. Precision ladder, all inside the 2e-2 rel-err gate
(measured end-to-end 1.34e-2): output fp16 (host upcasts), weights/text
fp16, and the visual input fp8 e3m4 -- its ~1.3e-2 quantization error is
the budget spend that halves the biggest load (16.8 -> 8.4 MiB/core) and
halves visual SBUF, funding a third output staging buffer.

Dataflow per core, against the ~250us DMA floor:
  * All loads are issued eagerly as a handful of large DMAs (two packed
    weight/bias images, text, 4 fp8 visual windows) and complete in the
    first ~35us at full rate, before the store stream ramps.
  * Output stores are grouped GRP=8 d_model chunks -> one 4 MiB DMA per
    group (4 per half-Lv window) on the Sync HWDGE ring; 3 staging buffers
    so a group's evacuation never waits on a 2-groups-ago store completing.
  * Fine-grained interleave: one x-proj+scan chunk of the NEXT window is
    emitted after each of this window's store-groups (last chunk one group
    early), so the store queue always has a group in flight while the PE
    does A-phase work and y(w+1) is ready with no boundary stall.

The out-projection runs TRANSPOSED (Wo.T stationary, scan output y as the
fp16 moving operand); output tiles come out [d_model-chunk, time] into a
grouped DRAM layout the host un-permutes. Everything ds-sized is DUPLICATED
onto both 64-partition halves: gate and x-proj matmuls issue column-tiled
twins ((0,0)/(0,64), concurrent in the PE array), the scan runs on all 128
partitions, and the out-projection row-packs TWO d_model chunks as
(0,0)/(64,0) tile pairs that execute concurrently -- halving PE time per
output tile so the PE stays off the critical path even at the cold
(1.2 GHz) HAM clock.

PSUM evacuation (ScalarE/VectorE are the only PSUM readers) adds bias bo
and casts to fp16 in one op. Each psA/psB pair splits across BOTH engines
so they drain concurrently (a block-modulo split alternates them serially
and paces the PE at single-engine rate -- measured 2x slower); the tile
pattern runs 9:7 toward the faster ScalarE to balance VectorE's extra scan
load, with the doubled-engine slot at a group's FIRST set so the group's
final tile never waits on a serialized pair. One unified PSUM pool
(4 x [128,1024] = all 8 banks) serves x-proj and out-proj.
"""

import os
import sys

import ml_dtypes
import numpy as np

for _p in ("/opt/trn_rl_repo",):
    if _p not in sys.path and os.path.isdir(_p):
        sys.path.insert(0, _p)

import concourse.bass as bass  # noqa: E402
import concourse.tile as tile  # noqa: E402
from concourse import bacc, mybir  # noqa: E402
from concourse.bass_utils import run_bass_kernel_spmd  # noqa: E402

F32 = mybir.dt.float32
F8E3 = mybir.dt.float8e3
F32R = mybir.dt.float32r
FP16 = mybir.dt.float16

# Problem shapes (hardcoded per the contract).
B, Lv, Dv = 16, 4096, 1024
Lt, Dt = 128, 4096
DS, DM = 64, 4096
NCORES = 8
BPC = B // NCORES  # batches per core

MM_DTYPE = F32R  # kept for test-harness compat; the big matmuls run fp16

NJ = Dt // 128  # gate contraction chunks
ND = Dv // 128  # x-proj contraction chunks
NH = 2  # halves of Lv (pipeline window = one half)
HLEN = Lv // NH
NTH = HLEN // 512  # 512-wide time chunks per half
NMC = DM // 128  # out-proj d_model chunks
GRP = 8  # d_model chunks coalesced per output store (4 MiB per DMA)
NG = NMC // GRP  # store groups per window (== NTH for the 1:1 interleave)


def _build_program(mm_dtype=MM_DTYPE):
    nc = bacc.Bacc()
    AF = mybir.ActivationFunctionType
    OP = mybir.AluOpType

    # All weight/text tensors are host-packed into their on-chip layouts so
    # every load is one large DMA with contiguous per-partition rows.
    vis16 = nc.dram_tensor("vis16", [BPC, NH, 128, ND * HLEN], F8E3, kind="ExternalInput")
    # both batches' text concatenated along the free dim: [128, NJ, BPC*Lt]
    text16 = nc.dram_tensor("text16", [128, NJ, BPC * Lt], FP16, kind="ExternalInput")
    # packed weights: cols 0:ND*DS = Wx.T chunks, cols ND*DS: = Wo.T row-dup
    wxo16 = nc.dram_tensor("wxo16", [128, ND * DS + DM], FP16, kind="ExternalInput")
    wg16 = nc.dram_tensor("wg16", [128, NJ, 3, DS], FP16, kind="ExternalInput")
    # packed f32 per-partition columns: bd, -bd, bB, bC, bx, then bo (NMC cols)
    biasf = nc.dram_tensor("biasf", [128, 5 + NMC], F32, kind="ExternalInput")
    # grouped output: element (b, h, g, p, j*HLEN+t) =
    #   out_fp16(batch b, dm=(g*GRP+j)*128+p, time=h*HLEN+t)
    outG16 = nc.dram_tensor(
        "outG16", [BPC, NH, NG, 128, GRP * HLEN], FP16, kind="ExternalOutput"
    )

    with tile.TileContext(nc) as tc:
        with (
            tc.tile_pool(name="persist", bufs=1) as persist,
            tc.tile_pool(name="visb", bufs=4) as visb,
        ):
            vts = {}

            def load_vis(b, h):
                vt = visb.tile([128, ND * HLEN], F8E3, tag="v", name="vt")
                nc.scalar.dma_start(out=vt[:], in_=vis16[b, h])
                vts[(b, h)] = vt

            # ---- eager preloads: two packed DMAs replace eight tiny ones
            # so the DMA ramp is short and the ring never idles at t=0 ----
            wxo_sb = persist.tile([128, ND * DS + DM], FP16)
            nc.scalar.dma_start(out=wxo_sb[:], in_=wxo16[:])
            bf_sb = persist.tile([128, 5 + NMC], F32)
            nc.scalar.dma_start(out=bf_sb[:], in_=biasf[:])
            bd_sb = bf_sb[:, 0:1]
            nbd_sb = bf_sb[:, 1:2]
            bb_sb = bf_sb[:, 2:3]
            bc_sb = bf_sb[:, 3:4]
            bx_sb = bf_sb[:, 4:5]

            delta_sb = persist.tile([128, BPC], F32)
            a_sb = persist.tile([128, BPC], F32)
            bv_sb = persist.tile([128, BPC], F32)
            cv_sb = persist.tile([128, BPC], F32)
            db_sb = persist.tile([128, BPC], F32)
            cdb_sb = persist.tile([128, BPC], F32)
            cdbx_sb = persist.tile([128, BPC], F32)

            # ---- Phase 0: fused text-mean gate projections (fp16 PE) ----
            # Both batches' tokens ride in one moving operand (N=BPC*Lt); every
            # z is computed twice via column-tiled twin matmuls so the whole
            # gate chain lives on all 128 partitions (rows 64-127 copy 0-63),
            # which the scan and out-proj packing rely on. Temporaries live in
            # a scoped pool that frees before the big main-loop pools open.
            with (
                tc.tile_pool(name="gatep", bufs=1) as gatep,
                tc.tile_pool(name="psum0", bufs=1, space="PSUM") as psum0,
            ):
                wg_sb = gatep.tile([128, NJ, 3, DS], FP16)
                nc.scalar.dma_start(out=wg_sb[:], in_=wg16[:])
                ttc = gatep.tile([128, NJ, BPC * Lt], FP16)
                nc.scalar.dma_start(out=ttc[:], in_=text16[:])
                # visual for the first two windows + out-proj weights: queued
                # behind the gate inputs, in first-use order.
                load_vis(0, 0)
                load_vis(0, 1)

                zd_sb = gatep.tile([128, BPC], F32)
                zb_sb = gatep.tile([128, BPC], F32)
                zc_sb = gatep.tile([128, BPC], F32)
                zd_ps = psum0.tile([128, BPC * Lt], F32, tag="zd")
                zb_ps = psum0.tile([128, BPC * Lt], F32, tag="zb")
                zc_ps = psum0.tile([128, BPC * Lt], F32, tag="zc")
                for j in range(NJ):
                    for g, ps in enumerate((zd_ps, zb_ps, zc_ps)):
                        for lo in (0, DS):
                            nc.tensor.matmul(
                                ps[lo : lo + DS, :],
                                wg_sb[:, j, g, :],
                                ttc[:, j, :],
                                start=(j == 0),
                                stop=(j == NJ - 1),
                            )
                for b in range(BPC):
                    bsl = slice(b * Lt, (b + 1) * Lt)
                    # mean over Lt (1/Lt folded into wg16 on host)
                    nc.vector.reduce_sum(
                        zd_sb[:, b : b + 1], zd_ps[:, bsl], axis=mybir.AxisListType.X
                    )
                    nc.vector.reduce_sum(
                        zb_sb[:, b : b + 1], zb_ps[:, bsl], axis=mybir.AxisListType.X
                    )
                    nc.vector.reduce_sum(
                        zc_sb[:, b : b + 1], zc_ps[:, bsl], axis=mybir.AxisListType.X
                    )

                nc.scalar.activation(
                    delta_sb[:], zd_sb[:], AF.Sigmoid, bias=bd_sb[:, 0:1], scale=1.0
                )
                nc.scalar.activation(
                    a_sb[:], zd_sb[:], AF.Sigmoid, bias=nbd_sb[:, 0:1], scale=-1.0
                )
                nc.vector.tensor_scalar_add(bv_sb[:], zb_sb[:], bb_sb[:, 0:1])
                nc.vector.tensor_scalar_add(cv_sb[:], zc_sb[:], bc_sb[:, 0:1])
                nc.vector.tensor_mul(db_sb[:], delta_sb[:], bv_sb[:])
                # Fold the output gate C into the scan input: scanning
                # u'_t = C*delta*B*x_t yields y_t = C*h_t directly.
                nc.vector.tensor_mul(cdb_sb[:], db_sb[:], cv_sb[:])
                nc.vector.tensor_scalar_mul(cdbx_sb[:], cdb_sb[:], bx_sb[:, 0:1])

            # ---- Phases 1+2: x-proj + chunked scan (A), out-proj (B) ----
            with (
                tc.tile_pool(name="psall", bufs=4, space="PSUM") as psall,
                tc.tile_pool(name="ubp", bufs=4) as ubp,
                tc.tile_pool(name="ybp", bufs=2) as ybp,
                tc.tile_pool(name="abp", bufs=2) as abp,
                tc.tile_pool(name="outp", bufs=3) as outp,
            ):
                ys, abcs = {}, {}

                def phase_a_begin(b, h):
                    if h == 0:
                        # broadcast decay gate a=(1-delta); the scan consumes
                        # the same [128, 512] columns every chunk.
                        a_bc = abp.tile([128, 512], F32, tag="a", name="a_bc")
                        nc.gpsimd.memset(a_bc[:], 1.0)
                        nc.vector.tensor_scalar_mul(
                            a_bc[:], a_bc[:], a_sb[:, b : b + 1]
                        )
                        abcs[b] = a_bc
                    ys[(b, h)] = ybp.tile([128, HLEN], FP16, tag="y", name="y_r")

                def phase_a_chunk(b, h, i):
                    y_r, a_bc, vt = ys[(b, h)], abcs[b], vts[(b, h)]
                    sl = slice(i * 512, (i + 1) * 512)
                    xp = psall.tile([128, 1024], F32, tag="op", name="xp")
                    for d in range(ND):
                        for lo in (0, DS):  # col-tiled twins, concurrent
                            nc.tensor.matmul(
                                xp[lo : lo + DS, 0:512],
                                wxo_sb[:, d * DS : (d + 1) * DS],
                                vt[:, d * HLEN + i * 512 : d * HLEN + (i + 1) * 512],
                                start=(d == 0),
                                stop=(d == ND - 1),
                            )
                    # u = (C*deltaB) * x_raw + (C*deltaB)*bx
                    u_c = ubp.tile([128, 512], F32, tag="u", name="u_c")
                    nc.scalar.activation(
                        u_c[:],
                        xp[:, 0:512],
                        AF.Identity,
                        bias=cdbx_sb[:, b : b + 1],
                        scale=cdb_sb[:, b : b + 1],
                    )
                    # chunked scan; chain via the previous chunk's last col
                    if i == 0:
                        init = 0.0 if h == 0 else ys[(b, 0)][:, HLEN - 1 : HLEN]
                    else:
                        init = y_r[:, i * 512 - 1 : i * 512]
                    nc.vector.tensor_tensor_scan(
                        y_r[:, sl],
                        a_bc[:],
                        u_c[:],
                        init,
                        OP.mult,
                        OP.add,
                    )

                def phase_a(b, h):
                    phase_a_begin(b, h)
                    for i in range(NTH):
                        phase_a_chunk(b, h, i)

                # PSUM evacuation (f32->fp16 + bias bo), fine-grained tile
                # interleave across ScalarE/VectorE so both engines drain PSUM
                # concurrently. 9:7 toward the faster ScalarE (0.93 vs 1.07
                # ns/elem) balances against VectorE's extra scan load; the
                # double-S lands at a group's FIRST set so the group-final
                # tile never waits on a serialized pair.
                EVAC_PAT = [1,1,0,1,0,1,0,1,0,1,0,1,0,1,0,1]  # 1=ScalarE
                evac_i = [0]

                def evac(ps, dst, mc):
                    if EVAC_PAT[evac_i[0] % 16]:
                        nc.scalar.activation(
                            dst, ps[:], AF.Identity,
                            bias=bf_sb[:, 5 + mc : 6 + mc], scale=1.0,
                        )
                    else:
                        nc.vector.tensor_scalar_add(
                            dst, ps[:], bf_sb[:, 5 + mc : 6 + mc]
                        )
                    evac_i[0] += 1

                def phase_b_group(b, h, g):
                    y_r = ys[(b, h)]
                    og = outp.tile([128, GRP * HLEN], FP16, tag="og", name="og")
                    for p in range(GRP // 2):
                        mcA = g * GRP + 2 * p
                        mcB = g * GRP + 2 * p + 1
                        for q in range(HLEN // 1024):
                            psA = psall.tile([128, 1024], F32, tag="op", name="psA")
                            psB = psall.tile([128, 1024], F32, tag="op", name="psB")
                            for hh in range(2):
                                sl = slice(q * 1024 + hh * 512, q * 1024 + (hh + 1) * 512)
                                osl = slice(hh * 512, (hh + 1) * 512)
                                # row-packed pair: (0,0) and (64,0) tiles run
                                # concurrently; each LDW overlaps the other
                                # row-group's running matmul.
                                nc.tensor.matmul(
                                    psA[:, osl],
                                    wxo_sb[0:DS, ND * DS + mcA * 128 : ND * DS + (mcA + 1) * 128],
                                    y_r[0:DS, sl],
                                    start=True,
                                    stop=True,
                                )
                                nc.tensor.matmul(
                                    psB[:, osl],
                                    wxo_sb[DS:128, ND * DS + mcB * 128 : ND * DS + (mcB + 1) * 128],
                                    y_r[DS:128, sl],
                                    start=True,
                                    stop=True,
                                )
                            base = 2 * p * HLEN + q * 1024
                            evac(psA, og[:, base : base + 1024], mcA)
                            evac(psB, og[:, base + HLEN : base + HLEN + 1024], mcB)
                    nc.sync.dma_start(out=outG16[b, h, g], in_=og[:])

                # Fine-grained interleave: one x-proj/scan chunk of the NEXT
                # window rides after every other one of this window's 2 MiB
                # store-groups, so the store queue always has groups in flight
                # while the PE does phase-A work. The last chunk is emitted
                # two groups early so y(w+1) completes while groups NG-2/NG-1
                # are still storing.
                windows = [(0, 0), (0, 1), (1, 0), (1, 1)]
                load_vis(1, 0)
                load_vis(1, 1)
                phase_a(0, 0)
                for w, (b, h) in enumerate(windows):
                    nxt = windows[w + 1] if w + 1 < len(windows) else None
                    if nxt:
                        phase_a_begin(*nxt)
                    for g in range(NG):
                        phase_b_group(b, h, g)
                        if nxt:
                            if g < NG - 2:
                                phase_a_chunk(*nxt, g)
                            elif g == NG - 2:
                                phase_a_chunk(*nxt, NG - 2)
                                phase_a_chunk(*nxt, NG - 1)
    return nc


def _prep_host_inputs(
    visual_feats, text_embeds, Wx, bx, Wd, bd, WB, bB, WC, bC, Wo, bo
):
    f = lambda a: np.asarray(a, dtype=np.float32)
    # [B, Lv, Dv] -> [B, NH, 128p, ND*HLEN] fp16 with element
    # (b, h, p, d*HLEN+t) = visual[b, h*HLEN+t, d*128+p]
    vis16 = np.ascontiguousarray(
        f(visual_feats)
        .transpose(0, 2, 1)
        .reshape(B, ND, 128, NH, HLEN)
        .transpose(0, 3, 2, 1, 4)
        .reshape(B, NH, 128, ND * HLEN)
        .astype(ml_dtypes.float8_e3m4)
    )
    # [B, Lt, Dt] -> per-core [128p, NJ, BPC*Lt] with Dt index = j*128 + p
    text16 = np.ascontiguousarray(
        f(text_embeds)
        .transpose(0, 2, 1)
        .reshape(B, NJ, 128, Lt)
        .transpose(0, 2, 1, 3)
        .astype(np.float16)
    )
    # Wx.T [Dv, ds] -> [128p, ND*ds] with Dv index = c*128 + p, then Wo.T
    # row-dup appended: one packed fp16 weight image, one DMA on device.
    wxt16 = (
        f(Wx).T.reshape(ND, 128, DS).transpose(1, 0, 2).reshape(128, ND * DS)
        .astype(np.float16)
    )
    # Gate weights transposed, pre-scaled by 1/Lt (the text mean), fp16,
    # packed [Dt, 3, ds] -> [128p, NJ, 3, ds] with Dt index = j*128 + p.
    wg16 = np.ascontiguousarray(
        (np.stack([f(Wd).T, f(WB).T, f(WC).T], axis=1) / np.float32(Lt))
        .reshape(NJ, 128, 3, DS)
        .transpose(1, 0, 2, 3)
        .astype(np.float16)
    )
    wot = f(Wo).T.astype(np.float16)  # [ds, dm]
    wott16 = np.concatenate([wot, wot], axis=0)  # [128, dm]
    wxo16 = np.ascontiguousarray(np.concatenate([wxt16, wott16], axis=1))
    # bo -> [128p, NMC] with dm index = mc*128 + p
    bo_t = f(bo).reshape(NMC, 128).T
    dup = lambda a: np.concatenate([f(a).reshape(-1, 1)] * 2, axis=0)
    biasf = np.ascontiguousarray(
        np.concatenate(
            [dup(bd), dup(-f(bd)), dup(bB), dup(bC), dup(bx), bo_t], axis=1
        ).astype(np.float32)
    )
    shared = {
        "wxo16": wxo16,
        "wg16": wg16,
        "biasf": biasf,
    }
    in_maps = []
    for c in range(NCORES):
        m = dict(shared)
        m["vis16"] = np.ascontiguousarray(vis16[c * BPC : (c + 1) * BPC])
        # concat the core's batches along the token axis: [128, NJ, BPC*Lt]
        m["text16"] = np.ascontiguousarray(
            np.concatenate(
                [text16[c * BPC + b] for b in range(BPC)], axis=2
            )
        )
        in_maps.append(m)
    return in_maps


_PROGRAM_CACHE = {}


def _get_program(mm_dtype=MM_DTYPE):
    key = str(mm_dtype)
    if key not in _PROGRAM_CACHE:
        nc = _build_program(mm_dtype)
        if not nc.is_finalized():
            nc.finalize()
        _PROGRAM_CACHE[key] = nc
    return _PROGRAM_CACHE[key]


def run(inputs: dict, trace: bool = False, mm_dtype=MM_DTYPE):
    """Run the kernel on all 8 cores; returns (full_output, BassKernelResults)."""
    nc = _get_program(mm_dtype)
    in_maps = _prep_host_inputs(**inputs)
    res = run_bass_kernel_spmd(nc, in_maps, list(range(NCORES)), trace=trace)
    # outG16 is [BPC, NH, NG, 128, GRP*HLEN] fp16 per core; un-permute + upcast
    # on host: (b, h, g, p, j, t) -> (b, (h,t), (g,j,p)).
    full = np.concatenate(
        [
            res.results[i]["outG16"]
            .reshape(BPC, NH, NG, 128, GRP, HLEN)
            .transpose(0, 1, 5, 2, 4, 3)
            .reshape(BPC, Lv, DM)
            .astype(np.float32)
            for i in range(NCORES)
        ],
        axis=0,
    )
    return np.ascontiguousarray(full), res


def kernel(**inputs) -> np.ndarray:
    out, _ = run(inputs, trace=False)
    return out
